# revision 17
# baseline (speedup 1.0000x reference)
"""Trainium2 Bass kernel for nn_Pointer_42021960024116 (topk_masking).

Reference computation (per step s of 4, teacher-forced):
    h   = GRUCell(ctx_in, h);  q_s = h @ Wq.T + bq
    e   = einsum('lba,a->lb', tanh(q_s + (context @ Wd.T + bd)), Ws[0]) + bs
    e   = e*m + (1-m)*NEG ; scores_s = softmax(e, axis=l)
    ctx_in, mask updated from prev_targets (teacher forcing).

Strategy:
  - Data-parallel over batch: 8 cores x 8 batch items.
  - The GRU/q recurrence is teacher-forced (independent of the attention
    output), tiny (<1% of FLOPs), and sequential -> computed on host.
    The device gets per-(step,batch) bias vectors  bd + q_s[b,:].
  - Context is uploaded pre-transposed (b, d, l) in bf16 so the device
    streams it straight into the PE array (contraction dim d on partitions).
  - Device per core:  d_ctx[a,l] = WdT.T @ ctxT   (PE, bf16)
                      tanh(d_ctx + bias)          (ACT, per-partition bias)
                      e[sb,l] += Wcol.T @ tanh    (PE; Wcol column-masked Ws
                                                   puts each (s,b) in its own
                                                   PSUM row of a (32,L) acc)
                      masked softmax over l on the (32,L) tile (DVE+ACT).
"""

import numpy as np
import ml_dtypes

L, B, D, H, A, S = 2048, 64, 512, 512, 512, 4
NCORES = 8
BL = B // NCORES  # batch per core
NEG = -1e8
BF16 = ml_dtypes.bfloat16

_NC_CACHE = {}


def _build_nc():
    import concourse.bacc as bacc
    import concourse.mybir as mybir
    import concourse.tile as tile

    f32 = mybir.dt.float32
    bf16 = mybir.dt.bfloat16
    AF = mybir.ActivationFunctionType

    nc = bacc.Bacc("TRN2", target_bir_lowering=False, debug=False,
                   num_devices=NCORES)

    ctxT_d = nc.dram_tensor("ctxT", [BL, 4, 128, L], bf16, kind="ExternalInput")
    wd_d = nc.dram_tensor("wdT", [128, 4 * A], bf16, kind="ExternalInput")
    wc_d = nc.dram_tensor("wcol", [4, 128, 32 * 32], bf16, kind="ExternalInput")
    bias_d = nc.dram_tensor("biasq", [128, 128], f32, kind="ExternalInput")
    ma_d = nc.dram_tensor("madd", [32, L], f32, kind="ExternalInput")
    eye_d = nc.dram_tensor("eye32", [32, 32], f32, kind="ExternalInput")
    out_d = nc.dram_tensor("scores", [32, L], f32, kind="ExternalOutput")

    with tile.TileContext(nc) as tc:
        with (
            tc.tile_pool(name="w", bufs=1) as wp,
            tc.tile_pool(name="ctx", bufs=16) as cp,
            tc.tile_pool(name="th", bufs=12) as tp,
            tc.tile_pool(name="sm", bufs=1) as sp,
            tc.tile_pool(name="sm2", bufs=3) as sp2,
            tc.tile_pool(name="dp", bufs=2, space="PSUM") as pp,
            tc.tile_pool(name="es", bufs=1, space="PSUM") as ep,
        ):
            # small operands needed by the very first matmul/ACT go first
            wd_all = wp.tile([128, 4 * A], bf16, tag="wdall")
            nc.sync.dma_start(wd_all[:], wd_d[:])
            wd = [wd_all[:, c * A:(c + 1) * A] for c in range(4)]
            bq_t = wp.tile([128, 128], f32, tag="bq")
            nc.sync.dma_start(bq_t[:], bias_d[:])
            ma_t = wp.tile([32, L], f32, tag="ma")
            nc.sync.dma_start(ma_t[:], ma_d[:])
            eye_t = wp.tile([32, 32], f32, tag="eye")
            nc.sync.dma_start(eye_t[:], eye_d[:])

            # context tiles: per (b, c, hh) quarters so the first
            # matmul only waits for 1MB, not 4MB
            def load_ct(b):
                out = {}
                for hh in range(2):
                    for c in range(4):
                        t = cp.tile([128, 1024], bf16, tag="ct",
                                    name=f"ct_{b}_{c}_{hh}")
                        nc.sync.dma_start(
                            t[:], ctxT_d[b, c, :, hh * 1024:(hh + 1) * 1024])
                        out[(c, hh)] = t
                return out

            ct = load_ct(0)

            wc = []
            for m in range(4):
                t = wp.tile([128, 32 * 32], bf16, tag=f"wc{m}")
                nc.sync.dma_start(t[:], wc_d[m])
                wc.append(t)
            es = ep.tile([32, L], f32)  # persistent (s,b)-row accumulator

            def emit_seeds():
                # seed each es region with the additive mask term
                # (I32.T @ madd) as the accumulation-group start; emitted
                # after the first d_ctx fill so it doesn't delay the first
                # ACT, but before any tanh-reduce matmul.
                for r in range(4):
                    nc.tensor.matmul(
                        es[:, r * 512:(r + 1) * 512], eye_t[:],
                        ma_t[:, r * 512:(r + 1) * 512],
                        start=True, stop=False, skip_group_check=True)

            # es accumulation-group bookkeeping: per 512-wide l-region,
            # start on first write, stop on last.
            es_started = [True] * 4

            def tanh_and_reduce(b, m, lo, width, src_ap, ths):
                """tanh(src + bias) for all 4 steps, then the Ws-weighted
                partition reduce into es rows via column-masked matmuls."""
                for s in range(S):
                    sb = s * BL + b
                    th = ths[s]
                    nc.scalar.activation(
                        th[:, lo:lo + width], src_ap, AF.Tanh,
                        bias=bq_t[:, m * 32 + sb:m * 32 + sb + 1],
                    )
                    esp = (b == BL - 1 and m == 3 and s == S - 1)
                    for ns in range(width // 512):
                        r = lo // 512 + ns
                        nc.tensor.matmul(
                            es[:, r * 512:(r + 1) * 512],
                            wc[m][:, sb * 32:(sb + 1) * 32],
                            th[:, r * 512:(r + 1) * 512],
                            start=not es_started[r], stop=esp,
                            skip_group_check=True,
                        )
                        es_started[r] = True

            def fill_and_copy(b, m):
                dpS = sp2.tile([128, L], f32, tag="dpS", name=f"dpS_{b}_{m}")
                for hh in range(2):
                    dp = pp.tile([128, 1024], f32, tag="dp",
                                 name=f"dp_{b}_{m}_{hh}")
                    for ns in range(2):
                        lsl = slice(ns * 512, (ns + 1) * 512)
                        for c in range(4):
                            nc.tensor.matmul(
                                dp[:, ns * 512:(ns + 1) * 512],
                                wd_all[:, c * A + m * 128:c * A + (m + 1) * 128],
                                ct[(c, hh)][:, lsl],
                                start=(c == 0), stop=(c == 3),
                            )
                    nc.vector.tensor_copy(
                        dpS[:, hh * 1024:(hh + 1) * 1024], dp[:])
                return dpS

            # software-pipelined emission: fill/copy of block k+1 precedes
            # tanh/reduce of block k in program order so the PE always has
            # the next d_ctx ready before ACT finishes the current one.
            blocks = [(b, m) for b in range(BL) for m in range(4)]
            DEPTH = 2
            pending = []
            for bi, (b, m) in enumerate(blocks):
                if bi == 0:
                    # head start: per-half FD=1024 ACTs so tanh begins as
                    # soon as the first l-half is staged
                    dpS = sp2.tile([128, L], f32, tag="dpS", name="dpS_hb")
                    ths0 = []
                    for s in range(S):
                        th = tp.tile([128, L], bf16, tag="th",
                                     name=f"th_h_{s}")
                        ths0.append(th)
                    for hh in range(2):
                        dp = pp.tile([128, 1024], f32, tag="dp",
                                     name=f"dp_h_{hh}")
                        for ns in range(2):
                            lsl = slice(ns * 512, (ns + 1) * 512)
                            for c in range(4):
                                nc.tensor.matmul(
                                    dp[:, ns * 512:(ns + 1) * 512],
                                    wd_all[:, c * A + m * 128:
                                           c * A + (m + 1) * 128],
                                    ct[(c, hh)][:, lsl],
                                    start=(c == 0), stop=(c == 3),
                                )
                        nc.vector.tensor_copy(
                            dpS[:, hh * 1024:(hh + 1) * 1024], dp[:])
                        if hh == 0:
                            emit_seeds()
                        tanh_and_reduce(
                            b, m, hh * 1024, 1024,
                            dpS[:, hh * 1024:(hh + 1) * 1024], ths0)
                    continue
                dpS = fill_and_copy(b, m)
                ths = []
                for s in range(S):
                    th = tp.tile([128, L], bf16, tag="th",
                                 name=f"th_{b}_{m}_{s}")
                    ths.append(th)
                pending.append((b, m, dpS, ths))
                if len(pending) > DEPTH:
                    pb, pm, pdpS, pths = pending.pop(0)
                    tanh_and_reduce(pb, pm, 0, L, pdpS[:], pths)
                if m == 3 and b + 1 < BL:
                    ct = load_ct(b + 1)
            for pb, pm, pdpS, pths in pending:
                tanh_and_reduce(pb, pm, 0, L, pdpS[:], pths)

            # softmax over l: es already holds e + bs + mask-NEG terms.
            # exp uses a constant shift C >= max(e) (softmax is shift-
            # invariant; exp(e-C) stays in safe fp32 range), so no
            # data-dependent row max is needed.
            zb = sp.tile([32, 1], f32, tag="zb")
            nc.vector.tensor_scalar_mul(zb[:], eye_t[:, 0:1], 0.0)
            ex = sp.tile([32, L], f32, tag="ex")
            sq = sp.tile([32, 4], f32, tag="sq")
            for q in range(4):
                nc.scalar.activation(ex[:, q * 512:(q + 1) * 512],
                                     es[:, q * 512:(q + 1) * 512], AF.Exp,
                                     bias=zb[:],
                                     accum_out=sq[:, q:q + 1])
            ssum = sp.tile([32, 1], f32, tag="ssum")
            nc.vector.tensor_reduce(ssum[:], sq[:],
                                    axis=mybir.AxisListType.X,
                                    op=mybir.AluOpType.add)
            rcp = sp.tile([32, 1], f32, tag="rcp")
            nc.vector.reciprocal(rcp[:], ssum[:])
            outt = sp.tile([32, L], f32, tag="outt")
            nc.vector.tensor_scalar_mul(outt[:], ex[:], rcp[:])
            nc.sync.dma_start(out_d[:], outt[:])

    nc.compile()
    return nc


def _sigmoid(x):
    return 1.0 / (1.0 + np.exp(-x))


def _ensure_axon_hooks_shim():
    """bass_utils' trace path imports antenv.axon_hooks, which this image
    lacks; provide a no-op shim so a stray BASS_TRACE=1 can't crash us."""
    try:
        import antenv.axon_hooks  # noqa: F401
    except ImportError:
        import sys
        import types
        import antenv
        mod = types.ModuleType("antenv.axon_hooks")
        _h = [None]
        mod.set_axon_ntff_profile_hook = lambda h: _h.__setitem__(0, h)
        mod.get_axon_ntff_profile_hook = lambda: _h[0]
        sys.modules["antenv.axon_hooks"] = mod
        antenv.axon_hooks = mod


def kernel(hidden, context, doc_sent_mask, pre_att_hard, prev_targets, max_step,
           W_ih, W_hh, b_ih, b_hh, Wq, bq, Wd, bd, Ws, bs):
    from concourse.bass_utils import run_bass_kernel_spmd
    _ensure_axon_hooks_shim()

    f = np.float32
    hidden = np.asarray(hidden, f)
    context = np.asarray(context, f)
    doc_sent_mask = np.asarray(doc_sent_mask, f)
    pre_att_hard = np.asarray(pre_att_hard, f)
    prev_targets = np.asarray(prev_targets)
    W_ih = np.asarray(W_ih, f); W_hh = np.asarray(W_hh, f)
    b_ih = np.asarray(b_ih, f); b_hh = np.asarray(b_hh, f)
    Wq = np.asarray(Wq, f); bq = np.asarray(bq, f)
    Wd = np.asarray(Wd, f); bd = np.asarray(bd, f)
    Ws = np.asarray(Ws, f); bs = np.asarray(bs, f)

    nsteps = prev_targets.shape[0]
    assert nsteps == S and context.shape == (L, B, D)

    # ---- host: teacher-forced GRU recurrence + q projections (tiny) ----
    h = hidden
    x = pre_att_hard
    qs = np.empty((S, B, A), f)
    for s in range(S):
        gi = x @ W_ih.T + b_ih
        gh = h @ W_hh.T + b_hh
        i_r, i_z, i_n = np.split(gi, 3, axis=-1)
        h_r, h_z, h_n = np.split(gh, 3, axis=-1)
        r = _sigmoid(i_r + h_r)
        z = _sigmoid(i_z + h_z)
        n = np.tanh(i_n + r * h_n)
        h = (1.0 - z) * n + z * h
        qs[s] = h @ Wq.T + bq
        x = context[prev_targets[s], np.arange(B)]

    # ---- host: per-step masks (teacher-forced updates) ----
    m = doc_sent_mask.copy()  # (B, L)
    mmul = np.empty((S, B, L), f)
    for s in range(S):
        mmul[s] = m
        m[np.arange(B), prev_targets[s]] = 0.0
    eclip = float(np.abs(Ws).sum() + np.abs(bs).sum() + 1.0)
    madd = (1.0 - mmul) * NEG + bs[0] - eclip

    # ---- host: device operand prep ----
    # context -> (B, D, L) bf16, per core (BL, 4, 128, L)
    ctxT = np.ascontiguousarray(context.transpose(1, 2, 0)).astype(BF16)
    ctxT = ctxT.reshape(B, 4, 128, L)
    wdT = np.ascontiguousarray(
        Wd.T.reshape(4, 128, A).transpose(1, 0, 2).reshape(128, 4 * A)
    ).astype(BF16)
    wcol = np.zeros((4, 128, 32 * 32), f)
    for mm_ in range(4):
        for sb in range(32):
            wcol[mm_, :, sb * 32 + sb] = Ws[0, mm_ * 128:(mm_ + 1) * 128]
    wcol = wcol.astype(BF16)

    in_maps = []
    for k in range(NCORES):
        b0 = k * BL
        biasq = np.empty((128, 128), f)
        for mm_ in range(4):
            for s in range(S):
                for b in range(BL):
                    biasq[:, mm_ * 32 + s * BL + b] = (
                        bd[mm_ * 128:(mm_ + 1) * 128]
                        + qs[s, b0 + b, mm_ * 128:(mm_ + 1) * 128])
        ma_core = np.empty((32, L), f)
        for s in range(S):
            for b in range(BL):
                ma_core[s * BL + b] = madd[s, b0 + b]
        in_maps.append({
            "ctxT": np.ascontiguousarray(ctxT[b0:b0 + BL]),
            "wdT": wdT,
            "wcol": wcol,
            "biasq": biasq,
            "madd": ma_core,
            "eye32": np.eye(32, dtype=f),
        })

    if "nc" not in _NC_CACHE:
        _NC_CACHE["nc"] = _build_nc()
    nc = _NC_CACHE["nc"]

    res = run_bass_kernel_spmd(nc, in_maps, core_ids=list(range(NCORES)))
    _NC_CACHE["last_result"] = res

    out = np.empty((S, L, B), f)
    for k in range(NCORES):
        o = res.results[k]["scores"]  # (32, L)
        for s in range(S):
            out[s, :, k * BL:(k + 1) * BL] = o[s * BL:(s + 1) * BL, :].T
    return out


# revision 18
# speedup vs baseline: 1.0364x; 1.0364x over previous
"""Trainium2 Bass kernel for nn_Pointer_42021960024116 (topk_masking).

Reference computation (per step s of 4, teacher-forced):
    h   = GRUCell(ctx_in, h);  q_s = h @ Wq.T + bq
    e   = einsum('lba,a->lb', tanh(q_s + (context @ Wd.T + bd)), Ws[0]) + bs
    e   = e*m + (1-m)*NEG ; scores_s = softmax(e, axis=l)
    ctx_in, mask updated from prev_targets (teacher forcing).

Strategy:
  - Data-parallel over batch: 8 cores x 8 batch items.
  - The GRU/q recurrence is teacher-forced (independent of the attention
    output), tiny (<1% of FLOPs), and sequential -> computed on host.
    The device gets per-(step,batch) bias vectors  bd + q_s[b,:].
  - Context is uploaded pre-transposed (b, d, l) in bf16 so the device
    streams it straight into the PE array (contraction dim d on partitions).
  - Device per core:  d_ctx[a,l] = WdT.T @ ctxT   (PE, bf16)
                      tanh(d_ctx + bias)          (ACT, per-partition bias)
                      e[sb,l] += Wcol.T @ tanh    (PE; Wcol column-masked Ws
                                                   puts each (s,b) in its own
                                                   PSUM row of a (32,L) acc)
                      masked softmax over l on the (32,L) tile (DVE+ACT).
"""

import numpy as np
import ml_dtypes

L, B, D, H, A, S = 2048, 64, 512, 512, 512, 4
NCORES = 8
BL = B // NCORES  # batch per core
NEG = -1e8
BF16 = ml_dtypes.bfloat16

_NC_CACHE = {}


def _build_nc():
    import concourse.bacc as bacc
    import concourse.mybir as mybir
    import concourse.tile as tile

    f32 = mybir.dt.float32
    bf16 = mybir.dt.bfloat16
    AF = mybir.ActivationFunctionType

    nc = bacc.Bacc("TRN2", target_bir_lowering=False, debug=False,
                   num_devices=NCORES)

    ctxT_d = nc.dram_tensor("ctxT", [BL, 4, 128, L], bf16, kind="ExternalInput")
    wd_d = nc.dram_tensor("wdT", [128, 4 * A], bf16, kind="ExternalInput")
    wc_d = nc.dram_tensor("wcol", [4, 128, 32 * 32], bf16, kind="ExternalInput")
    bias_d = nc.dram_tensor("biasq", [128, 128], f32, kind="ExternalInput")
    ma_d = nc.dram_tensor("madd", [32, L], bf16, kind="ExternalInput")
    eye_d = nc.dram_tensor("eye32", [32, 32], bf16, kind="ExternalInput")
    out_d = nc.dram_tensor("scores", [32, L], f32, kind="ExternalOutput")

    with tile.TileContext(nc) as tc:
        with (
            tc.tile_pool(name="w", bufs=1) as wp,
            tc.tile_pool(name="ctx", bufs=16) as cp,
            tc.tile_pool(name="th", bufs=12) as tp,
            tc.tile_pool(name="sm", bufs=1) as sp,
            tc.tile_pool(name="sm2", bufs=3) as sp2,
            tc.tile_pool(name="dp", bufs=2, space="PSUM") as pp,
            tc.tile_pool(name="es", bufs=1, space="PSUM") as ep,
        ):
            # small operands needed by the very first matmul/ACT go first
            wd_all = wp.tile([128, 4 * A], bf16, tag="wdall")
            nc.sync.dma_start(wd_all[:], wd_d[:])
            wd = [wd_all[:, c * A:(c + 1) * A] for c in range(4)]
            bq_t = wp.tile([128, 128], f32, tag="bq")
            nc.sync.dma_start(bq_t[:], bias_d[:])
            ma_t = wp.tile([32, L], bf16, tag="ma")
            nc.sync.dma_start(ma_t[:], ma_d[:])
            eye_t = wp.tile([32, 32], bf16, tag="eye")
            nc.sync.dma_start(eye_t[:], eye_d[:])

            # context tiles: per (b, c, hh) quarters so the first
            # matmul only waits for 1MB, not 4MB
            def load_ct(b):
                out = {}
                for hh in range(2):
                    for c in range(4):
                        t = cp.tile([128, 1024], bf16, tag="ct",
                                    name=f"ct_{b}_{c}_{hh}")
                        nc.sync.dma_start(
                            t[:], ctxT_d[b, c, :, hh * 1024:(hh + 1) * 1024])
                        out[(c, hh)] = t
                return out

            ct = load_ct(0)

            wc = []
            for m in range(4):
                t = wp.tile([128, 32 * 32], bf16, tag=f"wc{m}")
                nc.sync.dma_start(t[:], wc_d[m])
                wc.append(t)
            es = ep.tile([32, L], f32)  # persistent (s,b)-row accumulator

            def emit_seeds():
                # seed each es region with the additive mask term
                # (I32.T @ madd) as the accumulation-group start; emitted
                # after the first d_ctx fill so it doesn't delay the first
                # ACT, but before any tanh-reduce matmul.
                for r in range(4):
                    nc.tensor.matmul(
                        es[:, r * 512:(r + 1) * 512], eye_t[:],
                        ma_t[:, r * 512:(r + 1) * 512],
                        start=True, stop=False, skip_group_check=True)

            # es accumulation-group bookkeeping: per 512-wide l-region,
            # start on first write, stop on last.
            es_started = [True] * 4

            def tanh_and_reduce(b, m, lo, width, src_ap, ths):
                """tanh(src + bias) for all 4 steps, then the Ws-weighted
                partition reduce into es rows via column-masked matmuls."""
                for s in range(S):
                    sb = s * BL + b
                    th = ths[s]
                    nc.scalar.activation(
                        th[:, lo:lo + width], src_ap, AF.Tanh,
                        bias=bq_t[:, m * 32 + sb:m * 32 + sb + 1],
                    )
                    esp = (b == BL - 1 and m == 3 and s == S - 1)
                    for ns in range(width // 512):
                        r = lo // 512 + ns
                        nc.tensor.matmul(
                            es[:, r * 512:(r + 1) * 512],
                            wc[m][:, sb * 32:(sb + 1) * 32],
                            th[:, r * 512:(r + 1) * 512],
                            start=not es_started[r], stop=esp,
                            skip_group_check=True,
                        )
                        es_started[r] = True

            def fill_and_copy(b, m):
                dpS = sp2.tile([128, L], f32, tag="dpS", name=f"dpS_{b}_{m}")
                for hh in range(2):
                    dp = pp.tile([128, 1024], f32, tag="dp",
                                 name=f"dp_{b}_{m}_{hh}")
                    for ns in range(2):
                        lsl = slice(ns * 512, (ns + 1) * 512)
                        for c in range(4):
                            nc.tensor.matmul(
                                dp[:, ns * 512:(ns + 1) * 512],
                                wd_all[:, c * A + m * 128:c * A + (m + 1) * 128],
                                ct[(c, hh)][:, lsl],
                                start=(c == 0), stop=(c == 3),
                            )
                    nc.vector.tensor_copy(
                        dpS[:, hh * 1024:(hh + 1) * 1024], dp[:])
                return dpS

            # software-pipelined emission: fill/copy of block k+1 precedes
            # tanh/reduce of block k in program order so the PE always has
            # the next d_ctx ready before ACT finishes the current one.
            blocks = [(b, m) for b in range(BL) for m in range(4)]
            DEPTH = 2
            pending = []
            for bi, (b, m) in enumerate(blocks):
                if bi == 0:
                    # head start: per-half FD=1024 ACTs so tanh begins as
                    # soon as the first l-half is staged; seeds and the
                    # second half's fill run on PE under the first ACTs
                    dpS = fill_and_copy(b, m)
                    emit_seeds()
                    ths0 = []
                    for s in range(S):
                        th = tp.tile([128, L], bf16, tag="th",
                                     name=f"th_h_{s}")
                        ths0.append(th)
                    for hh in range(2):
                        tanh_and_reduce(
                            b, m, hh * 1024, 1024,
                            dpS[:, hh * 1024:(hh + 1) * 1024], ths0)
                    continue
                dpS = fill_and_copy(b, m)
                ths = []
                for s in range(S):
                    th = tp.tile([128, L], bf16, tag="th",
                                 name=f"th_{b}_{m}_{s}")
                    ths.append(th)
                pending.append((b, m, dpS, ths))
                if len(pending) > DEPTH:
                    pb, pm, pdpS, pths = pending.pop(0)
                    tanh_and_reduce(pb, pm, 0, L, pdpS[:], pths)
                if m == 3 and b + 1 < BL:
                    ct = load_ct(b + 1)
            for pb, pm, pdpS, pths in pending:
                tanh_and_reduce(pb, pm, 0, L, pdpS[:], pths)

            # softmax over l: es already holds e + bs + mask-NEG terms.
            # exp uses a constant shift C >= max(e) (softmax is shift-
            # invariant; exp(e-C) stays in safe fp32 range), so no
            # data-dependent row max is needed.
            zb = sp.tile([32, 1], f32, tag="zb")
            nc.vector.tensor_scalar_mul(zb[:], eye_t[:, 0:1], 0.0)
            ex = sp.tile([32, L], f32, tag="ex")
            sq = sp.tile([32, 4], f32, tag="sq")
            for q in range(4):
                nc.scalar.activation(ex[:, q * 512:(q + 1) * 512],
                                     es[:, q * 512:(q + 1) * 512], AF.Exp,
                                     bias=zb[:],
                                     accum_out=sq[:, q:q + 1])
            ssum = sp.tile([32, 1], f32, tag="ssum")
            nc.vector.tensor_reduce(ssum[:], sq[:],
                                    axis=mybir.AxisListType.X,
                                    op=mybir.AluOpType.add)
            rcp = sp.tile([32, 1], f32, tag="rcp")
            nc.vector.reciprocal(rcp[:], ssum[:])
            outt = sp.tile([32, L], f32, tag="outt")
            nc.vector.tensor_scalar_mul(outt[:], ex[:], rcp[:])
            nc.sync.dma_start(out_d[:], outt[:])

    nc.compile()
    return nc


def _sigmoid(x):
    return 1.0 / (1.0 + np.exp(-x))


def _ensure_axon_hooks_shim():
    """bass_utils' trace path imports antenv.axon_hooks, which this image
    lacks; provide a no-op shim so a stray BASS_TRACE=1 can't crash us."""
    try:
        import antenv.axon_hooks  # noqa: F401
    except ImportError:
        import sys
        import types
        import antenv
        mod = types.ModuleType("antenv.axon_hooks")
        _h = [None]
        mod.set_axon_ntff_profile_hook = lambda h: _h.__setitem__(0, h)
        mod.get_axon_ntff_profile_hook = lambda: _h[0]
        sys.modules["antenv.axon_hooks"] = mod
        antenv.axon_hooks = mod


def kernel(hidden, context, doc_sent_mask, pre_att_hard, prev_targets, max_step,
           W_ih, W_hh, b_ih, b_hh, Wq, bq, Wd, bd, Ws, bs):
    from concourse.bass_utils import run_bass_kernel_spmd
    _ensure_axon_hooks_shim()

    f = np.float32
    hidden = np.asarray(hidden, f)
    context = np.asarray(context, f)
    doc_sent_mask = np.asarray(doc_sent_mask, f)
    pre_att_hard = np.asarray(pre_att_hard, f)
    prev_targets = np.asarray(prev_targets)
    W_ih = np.asarray(W_ih, f); W_hh = np.asarray(W_hh, f)
    b_ih = np.asarray(b_ih, f); b_hh = np.asarray(b_hh, f)
    Wq = np.asarray(Wq, f); bq = np.asarray(bq, f)
    Wd = np.asarray(Wd, f); bd = np.asarray(bd, f)
    Ws = np.asarray(Ws, f); bs = np.asarray(bs, f)

    nsteps = prev_targets.shape[0]
    assert nsteps == S and context.shape == (L, B, D)

    # ---- host: teacher-forced GRU recurrence + q projections (tiny) ----
    h = hidden
    x = pre_att_hard
    qs = np.empty((S, B, A), f)
    for s in range(S):
        gi = x @ W_ih.T + b_ih
        gh = h @ W_hh.T + b_hh
        i_r, i_z, i_n = np.split(gi, 3, axis=-1)
        h_r, h_z, h_n = np.split(gh, 3, axis=-1)
        r = _sigmoid(i_r + h_r)
        z = _sigmoid(i_z + h_z)
        n = np.tanh(i_n + r * h_n)
        h = (1.0 - z) * n + z * h
        qs[s] = h @ Wq.T + bq
        x = context[prev_targets[s], np.arange(B)]

    # ---- host: per-step masks (teacher-forced updates) ----
    m = doc_sent_mask.copy()  # (B, L)
    mmul = np.empty((S, B, L), f)
    for s in range(S):
        mmul[s] = m
        m[np.arange(B), prev_targets[s]] = 0.0
    eclip = float(np.abs(Ws).sum() + np.abs(bs).sum() + 1.0)
    madd = (1.0 - mmul) * NEG + bs[0] - eclip

    # ---- host: device operand prep ----
    # context -> (B, D, L) bf16, per core (BL, 4, 128, L)
    ctxT = np.ascontiguousarray(context.transpose(1, 2, 0)).astype(BF16)
    ctxT = ctxT.reshape(B, 4, 128, L)
    wdT = np.ascontiguousarray(
        Wd.T.reshape(4, 128, A).transpose(1, 0, 2).reshape(128, 4 * A)
    ).astype(BF16)
    wcol = np.zeros((4, 128, 32 * 32), f)
    for mm_ in range(4):
        for sb in range(32):
            wcol[mm_, :, sb * 32 + sb] = Ws[0, mm_ * 128:(mm_ + 1) * 128]
    wcol = wcol.astype(BF16)

    in_maps = []
    for k in range(NCORES):
        b0 = k * BL
        biasq = np.empty((128, 128), f)
        for mm_ in range(4):
            for s in range(S):
                for b in range(BL):
                    biasq[:, mm_ * 32 + s * BL + b] = (
                        bd[mm_ * 128:(mm_ + 1) * 128]
                        + qs[s, b0 + b, mm_ * 128:(mm_ + 1) * 128])
        ma_core = np.empty((32, L), f)
        for s in range(S):
            for b in range(BL):
                ma_core[s * BL + b] = madd[s, b0 + b]
        in_maps.append({
            "ctxT": np.ascontiguousarray(ctxT[b0:b0 + BL]),
            "wdT": wdT,
            "wcol": wcol,
            "biasq": biasq,
            "madd": ma_core.astype(BF16),
            "eye32": np.eye(32, dtype=f).astype(BF16),
        })

    if "nc" not in _NC_CACHE:
        _NC_CACHE["nc"] = _build_nc()
    nc = _NC_CACHE["nc"]

    res = run_bass_kernel_spmd(nc, in_maps, core_ids=list(range(NCORES)))
    _NC_CACHE["last_result"] = res

    out = np.empty((S, L, B), f)
    for k in range(NCORES):
        o = res.results[k]["scores"]  # (32, L)
        for s in range(S):
            out[s, :, k * BL:(k + 1) * BL] = o[s * BL:(s + 1) * BL, :].T
    return out
